# revision 15
# baseline (speedup 1.0000x reference)
"""AttnTopKPool Trainium2 kernel, v6: 4-slot column sums, adds spread
across GpSimd/Scalar/DVE so no engine exceeds ~65% occupancy.

reference:
    w_mean = mean(w, axis=1)          # [B, S, S] -> [B, S]
    idx    = top_k(w_mean, 16)        # [B, 16]
    out    = x[b, :, idx[b]]          # [B, F, 16]

Strategy (8 NeuronCores, batch-parallel, 4 batches each):
  - host: transpose x to x_t[b, s, f]; slice w and x_t per core.
  - device per batch (16 MiB of w streamed once at ~425 GB/s):
      * 16 uniform 1 MiB quarter loads [128, 2048] (4 slots x 4 fr,
        rows 512t+4p+fr) on the sync HWDGE queue.
      * acc = ((w0+w1)+w2)+w3 chunk adds: add1 on GpSimd (earliest
        landings, most slack), add2 on Scalar (ACTIVATE Copy with
        tensor bias), add3 on DVE right behind the w3 landings so the
        matmul gate is a single fast DVE op. This summation order and
        engine split were validated on hardware to reproduce the jax
        fp32 reference top-16 ranking on all 32 batches (near-tied
        column sums make the ranking sensitive to exact fp32 rounding;
        see sums_experiment.py). fp32r matmuls would be 4x faster but
        mis-rank two batches - verified, do not use.
      * column sums via 16 fp32 ones-matmuls (PSUM accumulated, 4 banks).
      * top-16 via DVE max8 / max_index / match_replace, two rounds,
        software-pipelined one batch behind; gather issue deferred to
        the end of the next batch's issue block so reg_loads waiting on
        the top-k never head-of-line-block an add queue.
      * gather: per index, reg_load into a register and issue a
        dynamic-offset DMA copying that 4 KiB row of x_t[b] straight
        DRAM->DRAM into the output row (scalar + gpsimd queues; the
        final batch also uses the then-idle sync queue).
  - out per core: [64, 1024] = (b_loc*16 + k, f); host reassembles.
"""

import numpy as np

B, F, S, K = 32, 1024, 2048, 16
N_CORES = 8
B_LOC = B // N_CORES  # 4
P = 128
MM_N = 512                 # fp32 moving-operand max / one PSUM bank
NQ = S // MM_N             # 4 psum column slices
FR = 4                     # w rows per partition in a slot
NEG = -3.0e38              # below any column sum

_cached_nc = None

# test-only knobs (harness leaves these at defaults)
TRACE = False
_last_results = None


def _build_nc():
    from concourse import bacc, bass, mybir, tile

    f32 = mybir.dt.float32
    u32 = mybir.dt.uint32

    nc = bacc.Bacc("TRN2", target_bir_lowering=False, debug=False)

    w_d = nc.dram_tensor("w", [B_LOC, S, S], f32, kind="ExternalInput")
    xt_d = nc.dram_tensor("xt", [B_LOC, S, F], f32, kind="ExternalInput")
    out_d = nc.dram_tensor("out", [B_LOC * K, F], f32, kind="ExternalOutput")

    w_rows = w_d[:].rearrange("b r s -> (b r) s")
    # quarter view: [16, 4, 128, 2048]; [x, fr] partition p holds row 512x+4p+fr
    w_q = w_rows.rearrange("(x p fr) s -> x fr p s", p=P, fr=FR)

    with tile.TileContext(nc) as tc:
        with (
            tc.tile_pool(name="qpool", bufs=5) as qpool,
            tc.tile_pool(name="smpool", bufs=2) as smpool,
            tc.tile_pool(name="pspool", bufs=2, space="PSUM") as pspool,
            tc.tile_pool(name="tk", bufs=1) as tk,
        ):
            ones = tk.tile([P, 1], f32)
            nc.vector.memset(ones[:], 1.0)

            def topk(b, sums):
                """Two-round top-16 on DVE; returns the index tiles."""
                gidx_a = tk.tile([1, 8], u32, name=f"gidxa{b}", tag="gidxa", bufs=2)
                gidx_b = tk.tile([1, 8], u32, name=f"gidxb{b}", tag="gidxb", bufs=2)
                m8a = tk.tile([1, 8], f32, name=f"m8a{b}", tag="m8a", bufs=2)
                m8b = tk.tile([1, 8], f32, name=f"m8b{b}", tag="m8b", bufs=2)
                nc.vector.max(m8a[:], sums[:])
                nc.vector.max_index(gidx_a[:], m8a[:], sums[:])
                nc.vector.match_replace(sums[:], m8a[:], sums[:], NEG)
                nc.vector.max(m8b[:], sums[:])
                nc.vector.max_index(gidx_b[:], m8b[:], sums[:])
                return gidx_a, gidx_b

            def gathers(b, gidx_a, gidx_b, last):
                """Dynamic-offset DMAs, one 4 KiB x_t row straight
                DRAM->DRAM per selected index."""

                def gather(k, gidx, eng, etype):
                    regs = nc.alloc_registers(name=f"ri{b}_{k}", engines=(etype,))
                    reg = list(regs)[0]
                    eng.reg_load(reg, gidx[0:1, k % 8 : k % 8 + 1])
                    val = eng.snap(reg, donate=True, min_val=0, max_val=S - 1)
                    eng.dma_start(
                        out_d[b * K + k : b * K + k + 1, :],
                        xt_d[b][bass.ds(val, 1), :],
                    )

                for k in range(K):
                    gidx = gidx_a if k < 8 else gidx_b
                    r = k % 3
                    if last and r == 1:
                        gather(k, gidx, nc.sync, mybir.EngineType.SP)
                    elif r == 2 or (last and r == 0):
                        gather(k, gidx, nc.gpsimd, mybir.EngineType.Pool)
                    else:
                        gather(k, gidx, nc.scalar, mybir.EngineType.Activation)

            prev = None  # (b, gidx_a, gidx_b) whose gathers are deferred
            for b in range(B_LOC):
                # --- stream w[b]: 16 x 1 MiB quarter loads, sync queue ---
                w0 = [
                    qpool.tile([P, S], f32, name=f"w0_{b}_{fr}", tag="w0", bufs=6)
                    for fr in range(FR)
                ]
                w1 = [
                    qpool.tile([P, S], f32, name=f"w1_{b}_{fr}", tag="w1")
                    for fr in range(FR)
                ]
                w2 = [
                    qpool.tile([P, S], f32, name=f"w2_{b}_{fr}", tag="w2")
                    for fr in range(FR)
                ]
                w3 = [
                    qpool.tile([P, S], f32, name=f"w3_{b}_{fr}", tag="w3")
                    for fr in range(FR)
                ]
                for fr in range(FR):
                    nc.sync.dma_start(w0[fr][:], w_q[4 * b + 0, fr])
                for fr in range(FR):
                    nc.sync.dma_start(w1[fr][:], w_q[4 * b + 1, fr])
                for fr in range(FR):
                    nc.sync.dma_start(w2[fr][:], w_q[4 * b + 2, fr])
                for fr in range(FR):
                    nc.sync.dma_start(w3[fr][:], w_q[4 * b + 3, fr])

                # previous batch's top-k runs in this batch's early stream
                # window while DVE is otherwise idle (its first add3 gate
                # is the w3 landings, ~3/4 through the window)
                if prev is not None:
                    pb, psums = prev
                    pga, pgb = topk(pb, psums)

                # --- chunk adds: acc = ((w0+w1)+w2)+w3, elementwise ---
                # whole quarters assigned per engine (f0/f2 DVE, f1/f3
                # GpSimd) so each chain stays on one queue with no
                # cross-engine hops; the last batch keeps f3 on DVE so the
                # final matmul gate is a fast 2.3us op, not a 4.5us one.
                def eng_for(fr):
                    if fr % 2 == 0 or (b == B_LOC - 1 and fr == 3):
                        return nc.vector
                    return nc.gpsimd

                for fr in range(FR):
                    eng_for(fr).tensor_add(w0[fr][:], w0[fr][:], w1[fr][:])
                for fr in range(FR):
                    eng_for(fr).tensor_add(w0[fr][:], w0[fr][:], w2[fr][:])
                for fr in range(FR):
                    eng_for(fr).tensor_add(w0[fr][:], w0[fr][:], w3[fr][:])

                ps = [
                    pspool.tile([1, MM_N], f32, name=f"ps{b}_{q}", tag=f"ps{q}")
                    for q in range(NQ)
                ]
                # single accumulation group per psum slice; WAW deps on the
                # psum AP keep the start=True matmul first
                for c in range(FR * NQ):
                    fr, q = c // NQ, c % NQ
                    nc.tensor.matmul(
                        ps[q][:],
                        ones[:],
                        w0[fr][:, q * MM_N : (q + 1) * MM_N],
                        start=(c < NQ),
                        stop=(c >= FR * NQ - NQ),
                    )

                # previous batch's gathers go behind this batch's adds on
                # the scalar/gpsimd queues: their reg_loads' top-k gate has
                # already cleared, so they fill the queues' idle middle
                # window without blocking anything
                if prev is not None:
                    gathers(pb, pga, pgb, last=False)

                # PSUM -> column sums in SBUF
                sums = smpool.tile([1, S], f32, name=f"sums{b}", tag="sums")
                for q in range(NQ):
                    nc.scalar.activation(
                        sums[:, q * MM_N : (q + 1) * MM_N],
                        ps[q][:],
                        mybir.ActivationFunctionType.Copy,
                    )
                prev = (b, sums)

            # last batch's top-k + gathers are the kernel tail
            pb, psums = prev
            pga, pgb = topk(pb, psums)
            gathers(pb, pga, pgb, last=True)

    nc.compile()
    return nc


def _get_nc():
    global _cached_nc
    if _cached_nc is None:
        _cached_nc = _build_nc()
    return _cached_nc


def kernel(x: np.ndarray, w: np.ndarray) -> np.ndarray:
    from concourse import bass_utils

    x = np.asarray(x, dtype=np.float32)
    w = np.asarray(w, dtype=np.float32)
    x_t = np.ascontiguousarray(x.transpose(0, 2, 1))  # [B, S, F]

    nc = _get_nc()
    in_maps = [
        {
            "w": np.ascontiguousarray(w[c * B_LOC : (c + 1) * B_LOC]),
            "xt": x_t[c * B_LOC : (c + 1) * B_LOC],
        }
        for c in range(N_CORES)
    ]
    res = bass_utils.run_bass_kernel_spmd(
        nc, in_maps, list(range(N_CORES)), trace=TRACE
    )
    global _last_results
    _last_results = res
    out = np.concatenate([res.results[c]["out"] for c in range(N_CORES)], axis=0)
    # [B*K, F] -> [B, K, F] -> [B, F, K]
    return np.ascontiguousarray(out.reshape(B, K, F).transpose(0, 2, 1))
